# revision 1
# baseline (speedup 1.0000x reference)
"""MultiHeadAttention forward on 8 Trainium2 NeuronCores.

Reference:  x:[2,2048,1024], fused QKV (W_qkv:[3072,1024]), 16 heads x d_k=64,
softmax(QK^T/8)V, output projection W_o:[1024,1024].

Sharding: core c handles batch b = c//4 and head group g = c%4 (heads
4g..4g+3, i.e. a 256-wide slice of the model dim).  Each core computes its
partial output-projection contribution out_partial = attn_slice @ W_o[:, sl].T
(shape [2048,1024]); the host sums the 4 partials per batch and adds b_o.

Device layouts (all host-prepped, transposed so that matmul contraction is
always the SBUF partition dim):
  xt      [1024, 2048]  = x[b].T
  wqkv_t  [1024,  768]  = per-core W_qkv rows, permuted [Q0..Q3|K0..K3|V0..V3].T
  wo_t    [ 256, 1024]  = W_o[:, 256g:256g+256].T
  b_qk    [ 128,    4]  column e = bias for e-block e (Q01,Q23,K01,K23)
  b_v     [ 128,  256]  v-bias broadcast across partitions
Output:
  out     [2048, 1024]  partial (pre-b_o) result for batch b
"""

import sys

sys.path.insert(0, "/opt/trn_rl_repo")

import ml_dtypes
import numpy as np

import concourse.bass as bass
import concourse.mybir as mybir
import concourse.tile as tile
from concourse import bacc

F32 = mybir.dt.float32
F32R = mybir.dt.float32r
BF16 = mybir.dt.bfloat16

D_MODEL = 1024
N_HEADS = 16
D_K = 64
B = 2
S = 2048
N_CORES = 8
HL = 4  # heads per core
D_SLICE = HL * D_K  # 256


def _r(ap):
    """Matmul operands are stored as float32r already."""
    return ap


def build_kernel():
    nc = bacc.Bacc("TRN2")

    xt = nc.dram_tensor("xt", [D_MODEL, S], BF16, kind="ExternalInput")
    wqkv_t = nc.dram_tensor("wqkv_t", [D_MODEL, 3 * D_SLICE], BF16, kind="ExternalInput")
    wo_t = nc.dram_tensor("wo_t", [D_SLICE, D_MODEL], BF16, kind="ExternalInput")
    b_qk = nc.dram_tensor("b_qk", [128, 4], F32, kind="ExternalInput")
    b_v = nc.dram_tensor("b_v", [128, D_SLICE], F32, kind="ExternalInput")
    out = nc.dram_tensor("out", [S, D_MODEL], F32, kind="ExternalOutput")

    DC = D_MODEL // 128  # 8 contraction chunks for the QKV projection
    NT512 = S // 512  # 4
    NT128 = S // 128  # 16

    with tile.TileContext(nc) as tc:
        with tc.tile_pool(name="persist", bufs=1) as pp:
            # ---- persistent SBUF tensors ----
            xt_sb = [pp.tile([128, S], BF16, name=f"xt{i}", tag=f"xt{i}") for i in range(DC)]
            wq_sb = [pp.tile([128, 3 * D_SLICE], BF16, name=f"wq{i}", tag=f"wq{i}") for i in range(DC)]
            wo_sb = [pp.tile([128, D_MODEL], BF16, name=f"wo{i}", tag=f"wo{i}") for i in range(2)]
            bqk_sb = pp.tile([128, 4], F32, name="bqk", tag="bqk")
            bv_sb = pp.tile([128, D_SLICE], F32, name="bv", tag="bv")
            ones_sb = pp.tile([128, 1], F32, name="ones", tag="ones")
            # qk_sb[0]=Q heads01, [1]=Q heads23, [2]=K heads01, [3]=K heads23
            qk_sb = [pp.tile([128, S], BF16, name=f"qk{i}", tag=f"qk{i}") for i in range(4)]
            # v_sb[j]: seq tile j, 4 head blocks of 65 cols: [V_h (64) | ones]
            v_sb = [pp.tile([128, HL * 65], BF16, name=f"v{j}", tag=f"v{j}") for j in range(NT128)]
            # ot_sb[hp]: attention output^T, heads (2hp, 2hp+1) stacked
            ot_sb = [pp.tile([128, S], BF16, name=f"ot{i}", tag=f"ot{i}") for i in range(2)]

            # ---- small DMAs first ----
            nc.sync.dma_start(bqk_sb[:], b_qk[:])
            nc.sync.dma_start(bv_sb[:], b_v[:])
            nc.vector.memset(ones_sb[:], 1.0)
            # interleave weight chunks with the first x column block so the
            # first projection matmul can start as early as possible
            for i in range(DC):
                nc.sync.dma_start(wq_sb[i][:], wqkv_t[128 * i : 128 * (i + 1), :])
                nc.sync.dma_start(xt_sb[i][:, 0:512], xt[128 * i : 128 * (i + 1), 0:512])
            for i in range(2):
                nc.sync.dma_start(wo_sb[i][:], wo_t[128 * i : 128 * (i + 1), :])

            with tc.tile_pool(name="psum", bufs=2, space="PSUM") as ps_pool, \
                 tc.tile_pool(name="work", bufs=2) as wk_pool, \
                 tc.tile_pool(name="den", bufs=2) as dn_pool, \
                 tc.tile_pool(name="unnorm", bufs=6) as un_pool:

                # ====== Phase 1 helper: project one 512-wide column block ==
                def project_cb(cb):
                    cs = slice(512 * cb, 512 * (cb + 1))
                    if cb > 0:
                        for dc in range(DC):
                            nc.sync.dma_start(xt_sb[dc][:, cs], xt[128 * dc : 128 * (dc + 1), cs])
                    for eb in (0, 2, 1, 3):
                        ps = ps_pool.tile([128, 512], F32, name="pq", tag="pj", bufs=2)
                        for dc in range(DC):
                            nc.tensor.matmul(
                                ps[:],
                                wq_sb[dc][:, 128 * eb : 128 * (eb + 1)],
                                xt_sb[dc][:, cs],
                                start=(dc == 0),
                                stop=(dc == DC - 1),
                            )
                        with nc.allow_low_precision(reason="bf16 activations"):
                            nc.vector.tensor_scalar_add(
                                qk_sb[eb][:, cs], in0=ps[:], scalar1=bqk_sb[:, eb : eb + 1]
                            )
                    for jj in range(4):
                        j = 4 * cb + jj
                        ps = ps_pool.tile([128, 512], F32, name="pv", tag="pj", bufs=2)
                        psv = ps[:, 0:D_SLICE]
                        for dc in range(DC):
                            nc.tensor.matmul(
                                psv,
                                xt_sb[dc][:, 128 * j : 128 * (j + 1)],
                                wq_sb[dc][:, 2 * D_SLICE : 3 * D_SLICE],
                                start=(dc == 0),
                                stop=(dc == DC - 1),
                            )
                        vt = v_sb[j][:].rearrange("p (g x) -> p g x", x=65)
                        nc.vector.tensor_copy(
                            vt[:, :, 64:65],
                            ones_sb[:][:, None, :].broadcast_to((128, HL, 1)),
                        )
                        p3 = psv.rearrange("p (g x) -> p g x", x=64)
                        b3 = bv_sb[:].rearrange("p (g x) -> p g x", x=64)
                        with nc.allow_low_precision(reason="bf16 activations"):
                            nc.vector.tensor_add(vt[:, :, 0:64], p3, b3)

                # ====== Phase 2 helpers ======
                def attention_begin(st, hp):
                    pva = ps_pool.tile([65, 512], F32, name="pva", tag="pva", bufs=1)
                    pvb = ps_pool.tile([65, 512], F32, name="pvb", tag="pvb", bufs=1)
                    return {"st": st, "hp": hp, "pva": pva, "pvb": pvb}

                def attention_scores(a, kcs):
                    st, hp = a["st"], a["hp"]
                    qs = slice(512 * st, 512 * (st + 1))
                    q_t, k_t = qk_sb[hp], qk_sb[2 + hp]
                    for kc in kcs:
                        ks = slice(128 * kc, 128 * (kc + 1))
                        sc = ps_pool.tile([128, 1024], F32, name="sc", tag="sc", bufs=2)
                        nc.tensor.matmul(
                            sc[:, 0:512], k_t[0:64, ks], q_t[0:64, qs],
                            start=True, stop=True, tile_position=(0, 0),
                            skip_group_check=True,
                        )
                        nc.tensor.matmul(
                            sc[:, 512:1024], k_t[64:128, ks], q_t[64:128, qs],
                            start=True, stop=True, tile_position=(64, 0),
                            skip_group_check=True,
                        )
                        eab = wk_pool.tile([128, 1024], BF16, name="eab", tag="eab", bufs=44)
                        nc.scalar.activation(
                            eab[:], sc[:], mybir.ActivationFunctionType.Exp,
                            scale=0.125,
                        )
                        a.setdefault("eabs", {})[kc] = eab

                def attention_pvs(a, kcs):
                    hp = a["hp"]
                    for kc in kcs:
                        eab = a["eabs"].pop(kc)
                        for ph, pv_ps in ((0, a["pva"]), (1, a["pvb"])):
                            h = 2 * hp + ph
                            nc.tensor.matmul(
                                pv_ps[:],
                                v_sb[kc][:, 65 * h : 65 * h + 65],
                                eab[:, 512 * ph : 512 * (ph + 1)],
                                start=(kc == 0),
                                stop=(kc == NT128 - 1),
                                skip_group_check=True,
                            )

                def attention_kcs(a, kcs):
                    kcs = list(kcs)
                    attention_scores(a, kcs)
                    attention_pvs(a, kcs)

                def attention_finish(a):
                    st, hp = a["st"], a["hp"]
                    qs = slice(512 * st, 512 * (st + 1))
                    dr = dn_pool.tile([2, 512], F32, name="dr", tag="dr")
                    uns = []
                    for ph, pv_ps in ((0, a["pva"]), (1, a["pvb"])):
                        dstg = wk_pool.tile([1, 512], F32, name="dstg", tag="dstg")
                        nc.vector.tensor_copy(dstg[0:1, :], pv_ps[64:65, :])
                        nc.sync.dma_start(dr[ph : ph + 1, :], dstg[0:1, :])
                        un = un_pool.tile([64, 512], F32, name="un", tag="un")
                        nc.vector.tensor_copy(un[:], pv_ps[0:64, :])
                        uns.append(un)
                    rc = dn_pool.tile([2, 512], F32, name="rc", tag="rc")
                    nc.vector.reciprocal(rc[:], dr[:])
                    rc1 = dn_pool.tile([1, 512], F32, name="rc1", tag="rc1")
                    nc.sync.dma_start(rc1[0:1, :], rc[1:2, :])
                    for ph in range(2):
                        bc = dn_pool.tile([64, 512], F32, name="bc", tag="bc")
                        nc.gpsimd.partition_broadcast(
                            bc[:], rc[0:1, :] if ph == 0 else rc1[0:1, :]
                        )
                        with nc.allow_low_precision(reason="bf16 activations"):
                            nc.vector.tensor_mul(
                                ot_sb[hp][64 * ph : 64 * (ph + 1), qs],
                                uns[ph][:],
                                bc[:],
                            )

                def attention_block(st, hp):
                    a = attention_begin(st, hp)
                    attention_kcs(a, range(NT128))
                    attention_finish(a)

                def outproj_block(st):
                    for jj in range(4):
                        j = 4 * st + jj
                        js = slice(128 * j, 128 * (j + 1))
                        for nb in range(2):
                            ns = slice(512 * nb, 512 * (nb + 1))
                            po = ps_pool.tile([128, 512], F32, name="po", tag="pj", bufs=2)
                            for d2 in range(2):
                                nc.tensor.matmul(
                                    po[:],
                                    ot_sb[d2][:, js],
                                    wo_sb[d2][:, ns],
                                    start=(d2 == 0),
                                    stop=(d2 == 1),
                                )
                            ob = wk_pool.tile([128, 512], F32, name="ob", tag="ob")
                            nc.vector.tensor_copy(ob[:], po[:])
                            nc.sync.dma_start(out[js, ns], ob[:])

                # ====== interleaved emission: proj blocks feed attention ====
                project_cb(0)
                a00 = attention_begin(0, 0)
                a01 = attention_begin(0, 1)
                attention_kcs(a00, range(0, 4))
                attention_kcs(a01, range(0, 4))
                project_cb(1)
                attention_kcs(a00, range(4, 8))
                attention_kcs(a01, range(4, 8))
                a10 = attention_begin(1, 0)
                a11 = attention_begin(1, 1)
                attention_scores(a10, range(0, 4))
                attention_scores(a11, range(0, 4))
                project_cb(2)
                attention_kcs(a00, range(8, 12))
                attention_kcs(a01, range(8, 12))
                attention_scores(a10, range(4, 8))
                attention_scores(a11, range(4, 8))
                project_cb(3)
                attention_kcs(a00, range(12, 16))
                attention_kcs(a01, range(12, 16))
                attention_finish(a00)
                attention_finish(a01)
                attention_scores(a10, range(8, 16))
                attention_scores(a11, range(8, 16))
                a20 = attention_begin(2, 0)
                a21 = attention_begin(2, 1)
                attention_scores(a20, range(0, 4))
                attention_scores(a21, range(0, 4))
                attention_pvs(a10, range(0, 16))
                attention_finish(a10)
                attention_pvs(a11, range(0, 16))
                attention_finish(a11)
                outproj_block(0)
                attention_scores(a20, range(4, 16))
                attention_scores(a21, range(4, 16))
                a30 = attention_begin(3, 0)
                a31 = attention_begin(3, 1)
                attention_scores(a30, range(0, 4))
                attention_scores(a31, range(0, 4))
                attention_pvs(a20, range(0, 16))
                attention_finish(a20)
                attention_pvs(a21, range(0, 16))
                attention_finish(a21)
                outproj_block(1)
                attention_scores(a30, range(4, 16))
                attention_pvs(a30, range(0, 16))
                attention_finish(a30)
                attention_scores(a31, range(4, 16))
                attention_pvs(a31, range(0, 16))
                attention_finish(a31)
                outproj_block(2)
                outproj_block(3)

    nc.compile()
    return nc


def make_in_maps(x, W_qkv, b_qkv, W_o):
    """Per-core input dicts (host-side sharding + layout prep)."""
    x = np.asarray(x, np.float32)
    W_qkv = np.asarray(W_qkv, np.float32)
    b_qkv = np.asarray(b_qkv, np.float32)
    W_o = np.asarray(W_o, np.float32)

    in_maps = []
    xts = [np.ascontiguousarray(x[b].T).astype(ml_dtypes.bfloat16) for b in range(B)]
    for c in range(N_CORES):
        b, g = c // 4, c % 4
        heads = range(4 * g, 4 * g + 4)
        wq = [W_qkv[192 * h : 192 * h + 64] for h in heads]
        wk = [W_qkv[192 * h + 64 : 192 * h + 128] for h in heads]
        wv = [W_qkv[192 * h + 128 : 192 * h + 192] for h in heads]
        w_perm = np.concatenate(wq + wk + wv, axis=0)  # [768, 1024]
        bq = [b_qkv[192 * h : 192 * h + 64] for h in heads]
        bk = [b_qkv[192 * h + 64 : 192 * h + 128] for h in heads]
        bv = [b_qkv[192 * h + 128 : 192 * h + 192] for h in heads]
        b_perm = np.concatenate(bq + bk + bv)  # [768]
        in_maps.append(
            {
                "xt": xts[b],
                "wqkv_t": np.ascontiguousarray(w_perm.T).astype(ml_dtypes.bfloat16),
                "wo_t": np.ascontiguousarray(
                    W_o[:, 256 * g : 256 * g + 256].T
                ).astype(ml_dtypes.bfloat16),
                "b_qk": np.ascontiguousarray(b_perm[:512].reshape(4, 128).T),
                "b_v": np.ascontiguousarray(
                    np.broadcast_to(b_perm[512:], (128, 256))
                ),
            }
        )
    return in_maps


_NC = None


def kernel(x, W_qkv, b_qkv, W_o, b_o):
    global _NC
    from concourse.bass_utils import run_bass_kernel_spmd

    if _NC is None:
        _NC = build_kernel()
    in_maps = make_in_maps(x, W_qkv, b_qkv, W_o)
    res = run_bass_kernel_spmd(_NC, in_maps, core_ids=list(range(N_CORES)))
    b_o = np.asarray(b_o, np.float32)
    outs = [np.asarray(r["out"]) for r in res.results]
    full = np.empty((B, S, D_MODEL), np.float32)
    for b in range(B):
        full[b] = outs[4 * b] + outs[4 * b + 1] + outs[4 * b + 2] + outs[4 * b + 3]
        full[b] += b_o
    return full

